# revision 4
# baseline (speedup 1.0000x reference)
"""Sparse-weight matmul (BiologicalModule) on 8 Trainium2 NeuronCores.

Computes: out = tanh(x @ scatter_coo(kernel_vector, nonzero_ind) + bias)
  x [32, 30000] f32, 500K COO nonzeros into a [30000, 2048] weight matrix.

Strategy (units-sharded, 256 output columns per core):
  - Never materialize the dense [30000, 2048] weight matrix. In CSC view,
    out[b, c] = sum_k v[c,k] * x[r[c,k], b].
  - Host packs a padded-CSC payload with the entry-slot axis k on SBUF
    PARTITIONS: per core, per k-chunk kc, g[k_p, (slices of (b, c))] holds
    the x values each entry touches (fp16), v[k_p, c] the entry values.
    The bias is folded in as one extra entry slot per column (g=1,
    v=bias[c]), and the slot axis is cut exactly at max_count+1 (the last
    k-chunk has < 128 partition rows - no padding stream).
  - Device pipeline per column-slice:
      DVE : prod[k_p, (b,c)] = g * v  (v broadcast over b via a 0-stride
            middle dim; last dim step-1 fp16 keeps the 2x_1P perf mode)
      PE  : reduce over the partition (k) axis with ones-matmuls
            accumulated across the k-chunks in PSUM
            (out_mtile[128,1] = prod_tile[Pk,128m]^T @ ones[Pk,1])
      ACT : tanh(PSUM) -> SBUF f32, then DMA out.
  - DMA-in streams ~4 MB/core into two persistent per-kc g tiles in a few
    large chunks (descriptor-gen overhead off the critical path); compute
    slices are decoupled from DMA chunks via subtile dependencies. Column
    slices shrink toward the end so the post-stream tail is short.
"""

import sys

import numpy as np

_TRN_REPO = "/opt/trn_rl_repo"
if _TRN_REPO not in sys.path:
    sys.path.insert(0, _TRN_REPO)

INPUT_DIM = 30000
UNITS = 2048
BATCH = 32
N_CORES = 8
UPC = UNITS // N_CORES  # 256 columns per core
FREE = BATCH * UPC  # 8192 free elems per kc
# Column-slice widths (each 32*CS divisible by 128 -> CS % 4 == 0).
# Shrinking tail slices keep the post-stream critical path short.
SLICES = [48, 48, 48, 48, 48, 12, 4]
assert sum(SLICES) == UPC and all((BATCH * cs) % 128 == 0 for cs in SLICES)
# DMA chunk boundaries in the concatenated free dim (elems), per k-chunk.
CHUNKS = [(0, 3072), (3072, 3072), (6144, 1536), (7680, 512)]
assert sum(c[1] for c in CHUNKS) == FREE
# After which slice to flush the bulk of the outputs.
FLUSH_AFTER = 4

_PROGRAM_CACHE = {}


def _build_program(kp):
    """Build + compile the SPMD bass program for exact column length kp."""
    from concourse import bacc, tile
    from concourse.bass import AP
    import concourse.mybir as mybir

    f32 = mybir.dt.float32
    f16 = mybir.dt.float16
    nkc = -(-kp // 128)
    pkc = [min(128, kp - 128 * kc) for kc in range(nkc)]
    n_mt = FREE // 128  # total m-tiles (output columns of out_sb)

    nc = bacc.Bacc("TRN2", target_bir_lowering=False, debug=False,
                   num_devices=N_CORES)
    g_ds = [nc.dram_tensor(f"g{kc}", [pkc[kc], FREE], f16,
                           kind="ExternalInput") for kc in range(nkc)]
    v_d = nc.dram_tensor("vals", [128, nkc * UPC], f16, kind="ExternalInput")
    out_d = nc.dram_tensor("out", [128, n_mt], f32, kind="ExternalOutput")

    with tile.TileContext(nc) as tc:
        with (
            tc.tile_pool(name="persist", bufs=1) as persist,
            tc.tile_pool(name="pwork", bufs=4) as pwork,
            tc.psum_pool(name="psum", bufs=2) as psum,
        ):
            g_ts = [persist.tile([pkc[kc], FREE], f16, tag=f"g{kc}",
                                 name=f"g{kc}")
                    for kc in range(nkc)]
            # interleave kc0/kc1 chunk loads in compute-need order
            for off, sz in CHUNKS:
                for kc in range(nkc):
                    nc.sync.dma_start(g_ts[kc][:, off:off + sz],
                                      g_ds[kc][:, off:off + sz])
            v_t = persist.tile([128, nkc * UPC], f16, tag="v")
            nc.sync.dma_start(v_t[:], v_d[:])
            ones = persist.tile([128, 1], f16, tag="ones")
            nc.vector.memset(ones[:], 1.0)
            out_sb = persist.tile([128, n_mt], f32, tag="o")

            off = 0  # free-dim offset of current slice
            jo = 0  # m-tile offset of current slice
            for s, cs in enumerate(SLICES):
                fcs = BATCH * cs
                mt = fcs // 128
                prods = []
                for kc in range(nkc):
                    prod = pwork.tile([pkc[kc], fcs], f16, tag=f"prod{kc}",
                                      name=f"prod{s}_{kc}")
                    base = v_t[0:pkc[kc],
                               kc * UPC + off // BATCH:
                               kc * UPC + off // BATCH + cs]
                    v_bc = AP(base.tensor, base.offset,
                              [base.ap[0], [0, BATCH], base.ap[1]])
                    nc.vector.tensor_tensor(prod[:],
                                            g_ts[kc][:, off:off + fcs], v_bc,
                                            mybir.AluOpType.mult)
                    prods.append(prod)
                ps = psum.tile([128, mt], f32, tag="ps", name=f"ps{s}")
                for j in range(mt):
                    for kc in range(nkc):
                        nc.tensor.matmul(
                            ps[:, j:j + 1],
                            lhsT=prods[kc][:, 128 * j:128 * (j + 1)],
                            rhs=ones[0:pkc[kc], 0:1],
                            start=(kc == 0),
                            stop=(kc == nkc - 1),
                        )
                nc.scalar.activation(out_sb[:, jo:jo + mt], ps[:],
                                     mybir.ActivationFunctionType.Tanh)
                if s == FLUSH_AFTER:
                    nc.scalar.dma_start(out_d[:, 0:jo + mt],
                                        out_sb[:, 0:jo + mt])
                    flushed = jo + mt
                off += fcs
                jo += mt
            nc.scalar.dma_start(out_d[:, flushed:n_mt],
                                out_sb[:, flushed:n_mt])
    nc.compile()
    return nc


def _prepare(x, kernel_vector, bias, nonzero_ind):
    """Host-side shard prep. Returns (kp, per-core input dicts)."""
    x = np.asarray(x, dtype=np.float32)
    v = np.asarray(kernel_vector, dtype=np.float32).ravel()
    bias = np.asarray(bias, dtype=np.float32).ravel()
    ind = np.asarray(nonzero_ind)
    r = ind[:, 0].astype(np.int64)
    c = ind[:, 1].astype(np.int64)

    # COO .set semantics: de-duplicate (row, col), keeping the last occurrence.
    flat = r * UNITS + c
    if len(np.unique(flat)) != len(flat):
        _, last_rev = np.unique(flat[::-1], return_index=True)
        keep = np.sort(len(flat) - 1 - last_rev)
        r, c, v = r[keep], c[keep], v[keep]

    xt16 = np.ascontiguousarray(x.T).astype(np.float16)  # [INPUT_DIM, BATCH]

    # Sort by column, assign each entry its slot k within its column.
    order = np.argsort(c, kind="stable")
    r_s, c_s, v_s = r[order], c[order], v[order]
    counts = np.bincount(c_s, minlength=UNITS)
    kp = int(counts.max()) + 1  # +1: bias slot
    nkc = -(-kp // 128)
    starts = np.zeros(UNITS + 1, dtype=np.int64)
    np.cumsum(counts, out=starts[1:])
    k_s = np.arange(len(c_s), dtype=np.int64) - starts[c_s]

    # g_full[c, k, b]: x row for the entry at (column c, slot k); padding 0.
    g_full = np.zeros((UNITS, kp, BATCH), dtype=np.float16)
    g_full[c_s, k_s] = xt16[r_s]
    v_full = np.zeros((UNITS, kp), dtype=np.float16)
    v_full[c_s, k_s] = v_s.astype(np.float16)
    # bias as one extra entry: value bias[c], "x vector" of ones
    cols = np.arange(UNITS)
    g_full[cols, counts] = np.float16(1.0)
    v_full[cols, counts] = bias.astype(np.float16)

    g_full = g_full.reshape(N_CORES, UPC, kp, BATCH)
    v_full = v_full.reshape(N_CORES, UPC, kp)

    in_maps = []
    for d in range(N_CORES):
        m = {}
        for kc in range(nkc):
            p = min(128, kp - 128 * kc)
            parts = []
            co = 0
            for cs in SLICES:
                blk = g_full[d, co:co + cs, 128 * kc:128 * kc + p, :]
                # [cs, p, B] -> [p, B, cs] -> [p, B*cs]
                parts.append(blk.transpose(1, 2, 0).reshape(p, BATCH * cs))
                co += cs
            m[f"g{kc}"] = np.ascontiguousarray(np.concatenate(parts, axis=1))
        v_core = np.zeros((128, nkc * UPC), dtype=np.float16)
        for kc in range(nkc):
            p = min(128, kp - 128 * kc)
            v_core[:p, kc * UPC:kc * UPC + UPC] = \
                v_full[d, :, 128 * kc:128 * kc + p].T
        m["vals"] = v_core
        in_maps.append(m)
    return kp, in_maps


def _out_permutation():
    """Map out_sb[p, j] -> (b, c_local) per core.

    m-tile j of slice s covers flat f = (j - J_s)*128 + p of the (b-major,
    c_local-minor) flattening of [BATCH, CS_s]; slice s starts at column C_s.
    Returns (b_of, c_of) arrays of shape [128, n_mt].
    """
    n_mt = FREE // 128
    b_of = np.zeros((128, n_mt), dtype=np.int64)
    c_of = np.zeros((128, n_mt), dtype=np.int64)
    jo = 0
    co = 0
    p = np.arange(128)
    for cs in SLICES:
        mt = BATCH * cs // 128
        for j in range(mt):
            f = j * 128 + p
            b_of[:, jo + j] = f // cs
            c_of[:, jo + j] = co + f % cs
        jo += mt
        co += cs
    return b_of, c_of


def _unshard(res):
    b_of, c_of = _out_permutation()
    out = np.zeros((BATCH, UNITS), dtype=np.float32)
    for d in range(N_CORES):
        arr = np.asarray(res.results[d]["out"]).reshape(128, FREE // 128)
        out[b_of, UPC * d + c_of] = arr
    return out


def _run(inputs, trace=False):
    from concourse.bass_utils import run_bass_kernel_spmd

    kp, in_maps = _prepare(**inputs)
    if kp not in _PROGRAM_CACHE:
        _PROGRAM_CACHE[kp] = _build_program(kp)
    nc = _PROGRAM_CACHE[kp]
    res = None
    for attempt in range(3):
        try:
            res = run_bass_kernel_spmd(
                nc, in_maps, list(range(N_CORES)), trace=trace,
            )
            break
        except Exception:
            # Transient device faults (e.g. NRT_EXEC_UNIT_UNRECOVERABLE)
            # clear on re-execution; re-raise only if persistent.
            if attempt == 2:
                raise
    assert res is not None
    return _unshard(res), res


def kernel(**inputs):
    out, _ = _run(inputs, trace=False)
    return out


# revision 6
# speedup vs baseline: 1.4537x; 1.4537x over previous
"""Sparse-weight matmul (BiologicalModule) on 8 Trainium2 NeuronCores.

Computes: out = tanh(x @ scatter_coo(kernel_vector, nonzero_ind) + bias)
  x [32, 30000] f32, 500K COO nonzeros into a [30000, 2048] weight matrix.

Strategy (units-sharded, 256 output columns per core):
  - Never materialize the dense [30000, 2048] weight matrix. In CSC view,
    out[b, c] = sum_k v[c,k] * x[r[c,k], b].
  - Host packs a padded-CSC payload with the entry-slot axis k on SBUF
    PARTITIONS: per core, per k-chunk kc, g[k_p, (c, b)] holds the x values
    each entry touches (fp16) and v[k_p, c] the entry values. The bias is
    folded in as one extra entry slot per column (g=1, v=bias[c]), and the
    slot axis is cut exactly at max_count+1 (the last k-chunk has < 128
    partition rows - no padding in the stream).
  - For a single column the multiply+reduce IS one TensorE matmul:
        psum[32b, 1] (+)= g_col[Pk, 32b]^T @ v_col[Pk, 1]
    accumulated over the k-chunks in PSUM. The whole compute runs on the
    otherwise-idle PE engine; ACT applies tanh per chunk of columns
    (PSUM -> SBUF f32); DVE/GPSIMD do nothing.
  - DMA-in streams ~4 MB/core in a few large chunks; per-column matmuls
    are issued in chunk-arrival order (kc0 columns, then kc1 closing the
    PSUM accumulation), so all compute hides under the DMA stream. Column
    chunks shrink toward the end to keep the post-stream tail short.
"""

import sys

import numpy as np

_TRN_REPO = "/opt/trn_rl_repo"
if _TRN_REPO not in sys.path:
    sys.path.insert(0, _TRN_REPO)

INPUT_DIM = 30000
UNITS = 2048
BATCH = 32
N_CORES = 8
UPC = UNITS // N_CORES  # 256 columns per core
FREE = BATCH * UPC  # 8192 free elems per kc
# Columns per DMA chunk. >=8 keeps every DMA line >=512B; shrinking tail
# chunks keep the post-stream critical path short.
CHUNK_COLS = [64, 64, 64, 48, 8, 8]
assert sum(CHUNK_COLS) == UPC
# After which chunk to flush the bulk of the outputs.
FLUSH_AFTER = 4

_PROGRAM_CACHE = {}


def _build_program(kp):
    """Build + compile the SPMD bass program for exact column length kp."""
    from concourse import bacc, tile
    import concourse.mybir as mybir

    f32 = mybir.dt.float32
    f16 = mybir.dt.float16
    nkc = -(-kp // 128)
    pkc = [min(128, kp - 128 * kc) for kc in range(nkc)]

    nc = bacc.Bacc("TRN2", target_bir_lowering=False, debug=False,
                   num_devices=N_CORES)
    g_ds = [nc.dram_tensor(f"g{kc}", [pkc[kc], FREE], f16,
                           kind="ExternalInput") for kc in range(nkc)]
    v_d = nc.dram_tensor("vals", [128, nkc * UPC], f16, kind="ExternalInput")
    out_d = nc.dram_tensor("out", [BATCH, UPC], f32, kind="ExternalOutput")

    with tile.TileContext(nc) as tc:
        with (
            tc.tile_pool(name="persist", bufs=1) as persist,
            tc.tile_pool(name="gwork", bufs=3) as gwork,
            tc.psum_pool(name="psum", bufs=3) as psum,
        ):
            v_t = persist.tile([128, nkc * UPC], f16, tag="v")
            nc.sync.dma_start(v_t[:], v_d[:])
            out_sb = persist.tile([BATCH, UPC], f32, tag="o")

            co = 0
            for ci, w in enumerate(CHUNK_COLS):
                fo, fw = co * BATCH, w * BATCH
                g_ts = []
                for kc in range(nkc):
                    g_t = gwork.tile([pkc[kc], fw], f16, tag=f"g{kc}",
                                     name=f"g{ci}_{kc}")
                    nc.sync.dma_start(g_t[:], g_ds[kc][:, fo:fo + fw])
                    g_ts.append(g_t)
                ps = psum.tile([BATCH, w], f32, tag="ps", name=f"ps{ci}")
                # per-column matmuls; the k-chunk pair accumulates in PSUM
                for j in range(w):
                    for kc in range(nkc):
                        nc.tensor.matmul(
                            ps[:, j:j + 1],
                            lhsT=g_ts[kc][:, BATCH * j:BATCH * (j + 1)],
                            rhs=v_t[0:pkc[kc],
                                    kc * UPC + co + j:kc * UPC + co + j + 1],
                            start=(kc == 0),
                            stop=(kc == nkc - 1),
                        )
                nc.scalar.activation(out_sb[:, co:co + w], ps[:],
                                     mybir.ActivationFunctionType.Tanh)
                if ci == FLUSH_AFTER:
                    flushed = co + w
                    nc.scalar.dma_start(out_d[:, 0:flushed],
                                        out_sb[:, 0:flushed])
                co += w
            nc.scalar.dma_start(out_d[:, flushed:UPC],
                                out_sb[:, flushed:UPC])
    nc.compile()
    return nc


def _prepare(x, kernel_vector, bias, nonzero_ind):
    """Host-side shard prep. Returns (kp, per-core input dicts)."""
    x = np.asarray(x, dtype=np.float32)
    v = np.asarray(kernel_vector, dtype=np.float32).ravel()
    bias = np.asarray(bias, dtype=np.float32).ravel()
    ind = np.asarray(nonzero_ind)
    r = ind[:, 0].astype(np.int64)
    c = ind[:, 1].astype(np.int64)

    # COO .set semantics: de-duplicate (row, col), keeping the last occurrence.
    flat = r * UNITS + c
    if len(np.unique(flat)) != len(flat):
        _, last_rev = np.unique(flat[::-1], return_index=True)
        keep = np.sort(len(flat) - 1 - last_rev)
        r, c, v = r[keep], c[keep], v[keep]

    xt16 = np.ascontiguousarray(x.T).astype(np.float16)  # [INPUT_DIM, BATCH]

    # Sort by column, assign each entry its slot k within its column.
    order = np.argsort(c, kind="stable")
    r_s, c_s, v_s = r[order], c[order], v[order]
    counts = np.bincount(c_s, minlength=UNITS)
    kp = int(counts.max()) + 1  # +1: bias slot
    nkc = -(-kp // 128)
    starts = np.zeros(UNITS + 1, dtype=np.int64)
    np.cumsum(counts, out=starts[1:])
    k_s = np.arange(len(c_s), dtype=np.int64) - starts[c_s]

    # g_full[c, k, b]: x row for the entry at (column c, slot k); padding 0.
    g_full = np.zeros((UNITS, kp, BATCH), dtype=np.float16)
    g_full[c_s, k_s] = xt16[r_s]
    v_full = np.zeros((UNITS, kp), dtype=np.float16)
    v_full[c_s, k_s] = v_s.astype(np.float16)
    # bias as one extra entry: value bias[c], "x vector" of ones
    cols = np.arange(UNITS)
    g_full[cols, counts] = np.float16(1.0)
    v_full[cols, counts] = bias.astype(np.float16)

    g_full = g_full.reshape(N_CORES, UPC, kp, BATCH)
    v_full = v_full.reshape(N_CORES, UPC, kp)

    in_maps = []
    for d in range(N_CORES):
        m = {}
        for kc in range(nkc):
            p = min(128, kp - 128 * kc)
            # [UPC, p, B] -> [p, UPC, B] -> [p, UPC*B] (c-major free dim)
            blk = g_full[d, :, 128 * kc:128 * kc + p, :]
            m[f"g{kc}"] = np.ascontiguousarray(
                blk.transpose(1, 0, 2)).reshape(p, UPC * BATCH)
        v_core = np.zeros((128, nkc * UPC), dtype=np.float16)
        for kc in range(nkc):
            p = min(128, kp - 128 * kc)
            v_core[:p, kc * UPC:kc * UPC + UPC] = \
                v_full[d, :, 128 * kc:128 * kc + p].T
        m["vals"] = v_core
        in_maps.append(m)
    return kp, in_maps


def _unshard(res):
    out = np.concatenate(
        [np.asarray(res.results[d]["out"]).reshape(BATCH, UPC)
         for d in range(N_CORES)], axis=1)
    return np.ascontiguousarray(out).astype(np.float32)


def _run(inputs, trace=False):
    from concourse.bass_utils import run_bass_kernel_spmd

    kp, in_maps = _prepare(**inputs)
    if kp not in _PROGRAM_CACHE:
        _PROGRAM_CACHE[kp] = _build_program(kp)
    nc = _PROGRAM_CACHE[kp]
    res = None
    for attempt in range(3):
        try:
            res = run_bass_kernel_spmd(
                nc, in_maps, list(range(N_CORES)), trace=trace,
            )
            break
        except Exception:
            # Transient device faults (e.g. NRT_EXEC_UNIT_UNRECOVERABLE)
            # clear on re-execution; re-raise only if persistent.
            if attempt == 2:
                raise
    assert res is not None
    return _unshard(res), res


def kernel(**inputs):
    out, _ = _run(inputs, trace=False)
    return out


# revision 7
# speedup vs baseline: 1.4559x; 1.0016x over previous
"""Sparse-weight matmul (BiologicalModule) on 8 Trainium2 NeuronCores.

Computes: out = tanh(x @ scatter_coo(kernel_vector, nonzero_ind) + bias)
  x [32, 30000] f32, 500K COO nonzeros into a [30000, 2048] weight matrix.

Strategy (units-sharded, 256 output columns per core):
  - Never materialize the dense [30000, 2048] weight matrix. In CSC view,
    out[b, c] = sum_k v[c,k] * x[r[c,k], b].
  - Host packs a padded-CSC payload with the entry-slot axis k on SBUF
    PARTITIONS: per core, per k-chunk kc, g[k_p, (c, b)] holds the x values
    each entry touches (fp16) and v[k_p, c] the entry values. The bias is
    folded in as one extra entry slot per column (g=1, v=bias[c]), and the
    slot axis is cut exactly at max_count+1 (the last k-chunk has < 128
    partition rows - no padding in the stream).
  - For a single column the multiply+reduce IS one TensorE matmul:
        psum[32b, 1] (+)= g_col[Pk, 32b]^T @ v_col[Pk, 1]
    accumulated over the k-chunks in PSUM. The whole compute runs on the
    otherwise-idle PE engine; ACT applies tanh per chunk of columns
    (PSUM -> SBUF f32); DVE/GPSIMD do nothing.
  - DMA-in streams ~4 MB/core in a few large chunks; per-column matmuls
    are issued in chunk-arrival order (kc0 columns, then kc1 closing the
    PSUM accumulation), so all compute hides under the DMA stream. Column
    chunks shrink toward the end to keep the post-stream tail short.
"""

import sys

import numpy as np

_TRN_REPO = "/opt/trn_rl_repo"
if _TRN_REPO not in sys.path:
    sys.path.insert(0, _TRN_REPO)

INPUT_DIM = 30000
UNITS = 2048
BATCH = 32
N_CORES = 8
UPC = UNITS // N_CORES  # 256 columns per core
FREE = BATCH * UPC  # 8192 free elems per kc
# Columns per DMA chunk. >=8 keeps every DMA line >=512B; shrinking tail
# chunks keep the post-stream critical path short.
CHUNK_COLS = [64, 64, 64, 48, 8, 8]
assert sum(CHUNK_COLS) == UPC
# After which chunk to flush the bulk of the outputs.
FLUSH_AFTER = 4

_PROGRAM_CACHE = {}


def _build_program(kp):
    """Build + compile the SPMD bass program for exact column length kp."""
    from concourse import bacc, tile
    import concourse.mybir as mybir

    f32 = mybir.dt.float32
    f16 = mybir.dt.float16
    nkc = -(-kp // 128)
    pkc = [min(128, kp - 128 * kc) for kc in range(nkc)]

    nc = bacc.Bacc("TRN2", target_bir_lowering=False, debug=False,
                   num_devices=N_CORES)
    g_ds = [nc.dram_tensor(f"g{kc}", [pkc[kc], FREE], f16,
                           kind="ExternalInput") for kc in range(nkc)]
    v_d = nc.dram_tensor("vals", [128, nkc * UPC], f16, kind="ExternalInput")
    out_d = nc.dram_tensor("out", [BATCH, UPC], f32, kind="ExternalOutput")

    with tile.TileContext(nc) as tc:
        with (
            tc.tile_pool(name="persist", bufs=1) as persist,
            tc.tile_pool(name="gwork", bufs=6) as gwork,
            tc.psum_pool(name="psum", bufs=4) as psum,
        ):
            v_t = persist.tile([128, nkc * UPC], f16, tag="v")
            nc.sync.dma_start(v_t[:], v_d[:])
            out_sb = persist.tile([BATCH, UPC], f32, tag="o")

            co = 0
            for ci, w in enumerate(CHUNK_COLS):
                fo, fw = co * BATCH, w * BATCH
                g_ts = []
                for kc in range(nkc):
                    g_t = gwork.tile([pkc[kc], fw], f16, tag=f"g{kc}",
                                     name=f"g{ci}_{kc}")
                    nc.sync.dma_start(g_t[:], g_ds[kc][:, fo:fo + fw])
                    g_ts.append(g_t)
                ps = psum.tile([BATCH, w], f32, tag="ps", name=f"ps{ci}")
                # per-column matmuls; the k-chunk pair accumulates in PSUM
                for j in range(w):
                    for kc in range(nkc):
                        nc.tensor.matmul(
                            ps[:, j:j + 1],
                            lhsT=g_ts[kc][:, BATCH * j:BATCH * (j + 1)],
                            rhs=v_t[0:pkc[kc],
                                    kc * UPC + co + j:kc * UPC + co + j + 1],
                            start=(kc == 0),
                            stop=(kc == nkc - 1),
                        )
                nc.scalar.activation(out_sb[:, co:co + w], ps[:],
                                     mybir.ActivationFunctionType.Tanh)
                if ci == FLUSH_AFTER:
                    flushed = co + w
                    nc.scalar.dma_start(out_d[:, 0:flushed],
                                        out_sb[:, 0:flushed])
                co += w
            nc.scalar.dma_start(out_d[:, flushed:UPC],
                                out_sb[:, flushed:UPC])
    nc.compile()
    return nc


def _prepare(x, kernel_vector, bias, nonzero_ind):
    """Host-side shard prep. Returns (kp, per-core input dicts)."""
    x = np.asarray(x, dtype=np.float32)
    v = np.asarray(kernel_vector, dtype=np.float32).ravel()
    bias = np.asarray(bias, dtype=np.float32).ravel()
    ind = np.asarray(nonzero_ind)
    r = ind[:, 0].astype(np.int64)
    c = ind[:, 1].astype(np.int64)

    # COO .set semantics: de-duplicate (row, col), keeping the last occurrence.
    flat = r * UNITS + c
    if len(np.unique(flat)) != len(flat):
        _, last_rev = np.unique(flat[::-1], return_index=True)
        keep = np.sort(len(flat) - 1 - last_rev)
        r, c, v = r[keep], c[keep], v[keep]

    xt16 = np.ascontiguousarray(x.T).astype(np.float16)  # [INPUT_DIM, BATCH]

    # Sort by column, assign each entry its slot k within its column.
    order = np.argsort(c, kind="stable")
    r_s, c_s, v_s = r[order], c[order], v[order]
    counts = np.bincount(c_s, minlength=UNITS)
    kp = int(counts.max()) + 1  # +1: bias slot
    nkc = -(-kp // 128)
    starts = np.zeros(UNITS + 1, dtype=np.int64)
    np.cumsum(counts, out=starts[1:])
    k_s = np.arange(len(c_s), dtype=np.int64) - starts[c_s]

    # g_full[c, k, b]: x row for the entry at (column c, slot k); padding 0.
    g_full = np.zeros((UNITS, kp, BATCH), dtype=np.float16)
    g_full[c_s, k_s] = xt16[r_s]
    v_full = np.zeros((UNITS, kp), dtype=np.float16)
    v_full[c_s, k_s] = v_s.astype(np.float16)
    # bias as one extra entry: value bias[c], "x vector" of ones
    cols = np.arange(UNITS)
    g_full[cols, counts] = np.float16(1.0)
    v_full[cols, counts] = bias.astype(np.float16)

    g_full = g_full.reshape(N_CORES, UPC, kp, BATCH)
    v_full = v_full.reshape(N_CORES, UPC, kp)

    in_maps = []
    for d in range(N_CORES):
        m = {}
        for kc in range(nkc):
            p = min(128, kp - 128 * kc)
            # [UPC, p, B] -> [p, UPC, B] -> [p, UPC*B] (c-major free dim)
            blk = g_full[d, :, 128 * kc:128 * kc + p, :]
            m[f"g{kc}"] = np.ascontiguousarray(
                blk.transpose(1, 0, 2)).reshape(p, UPC * BATCH)
        v_core = np.zeros((128, nkc * UPC), dtype=np.float16)
        for kc in range(nkc):
            p = min(128, kp - 128 * kc)
            v_core[:p, kc * UPC:kc * UPC + UPC] = \
                v_full[d, :, 128 * kc:128 * kc + p].T
        m["vals"] = v_core
        in_maps.append(m)
    return kp, in_maps


def _unshard(res):
    out = np.concatenate(
        [np.asarray(res.results[d]["out"]).reshape(BATCH, UPC)
         for d in range(N_CORES)], axis=1)
    return np.ascontiguousarray(out).astype(np.float32)


def _run(inputs, trace=False):
    from concourse.bass_utils import run_bass_kernel_spmd

    kp, in_maps = _prepare(**inputs)
    if kp not in _PROGRAM_CACHE:
        _PROGRAM_CACHE[kp] = _build_program(kp)
    nc = _PROGRAM_CACHE[kp]
    res = None
    for attempt in range(3):
        try:
            res = run_bass_kernel_spmd(
                nc, in_maps, list(range(N_CORES)), trace=trace,
            )
            break
        except Exception:
            # Transient device faults (e.g. NRT_EXEC_UNIT_UNRECOVERABLE)
            # clear on re-execution; re-raise only if persistent.
            if attempt == 2:
                raise
    assert res is not None
    return _unshard(res), res


def kernel(**inputs):
    out, _ = _run(inputs, trace=False)
    return out


# revision 8
# speedup vs baseline: 1.5490x; 1.0639x over previous
"""Sparse-weight matmul (BiologicalModule) on 8 Trainium2 NeuronCores.

Computes: out = tanh(x @ scatter_coo(kernel_vector, nonzero_ind) + bias)
  x [32, 30000] f32, 500K COO nonzeros into a [30000, 2048] weight matrix.

Strategy (units-sharded, 256 output columns per core):
  - Never materialize the dense [30000, 2048] weight matrix. In CSC view,
    out[b, c] = sum_k v[c,k] * x[r[c,k], b].
  - Host packs a padded-CSC payload with the entry-slot axis k on SBUF
    PARTITIONS: per core, per k-chunk kc, g[k_p, (c, b)] holds the x values
    each entry touches (fp16) and v[k_p, c] the entry values. The bias is
    folded in as one extra entry slot per column (g=1, v=bias[c]), and the
    slot axis is cut exactly at max_count+1 (the last k-chunk has < 128
    partition rows - no padding in the stream).
  - For a single column the multiply+reduce IS one TensorE matmul:
        psum[32b, 1] (+)= g_col[Pk, 32b]^T @ v_col[Pk, 1]
    accumulated over the k-chunks in PSUM. The whole compute runs on the
    otherwise-idle PE engine; ACT applies tanh per chunk of columns
    (PSUM -> SBUF f32); DVE/GPSIMD do nothing.
  - DMA-in streams ~4 MB/core in a few large chunks; per-column matmuls
    are issued in chunk-arrival order (kc0 columns, then kc1 closing the
    PSUM accumulation), so all compute hides under the DMA stream. Column
    chunks shrink toward the end to keep the post-stream tail short.
"""

import sys

import numpy as np

_TRN_REPO = "/opt/trn_rl_repo"
if _TRN_REPO not in sys.path:
    sys.path.insert(0, _TRN_REPO)

INPUT_DIM = 30000
UNITS = 2048
BATCH = 32
N_CORES = 8
UPC = UNITS // N_CORES  # 256 columns per core
FREE = BATCH * UPC  # 8192 free elems per kc
# Columns per DMA chunk. >=8 keeps every DMA line >=512B; shrinking tail
# chunks keep the post-stream critical path short.
CHUNK_COLS = [64, 64, 64, 48, 8, 8]
assert sum(CHUNK_COLS) == UPC
# After which chunk to flush the bulk of the outputs.
FLUSH_AFTER = 2

_PROGRAM_CACHE = {}


def _build_program(kp):
    """Build + compile the SPMD bass program for exact column length kp."""
    from concourse import bacc, tile
    import concourse.mybir as mybir

    f32 = mybir.dt.float32
    f16 = mybir.dt.float16
    nkc = -(-kp // 128)
    pkc = [min(128, kp - 128 * kc) for kc in range(nkc)]

    nc = bacc.Bacc("TRN2", target_bir_lowering=False, debug=False,
                   num_devices=N_CORES)
    g_ds = [nc.dram_tensor(f"g{kc}", [pkc[kc], FREE], f16,
                           kind="ExternalInput") for kc in range(nkc)]
    v_d = nc.dram_tensor("vals", [128, nkc * UPC], f16, kind="ExternalInput")
    out_d = nc.dram_tensor("out", [BATCH, UPC], f32, kind="ExternalOutput")

    with tile.TileContext(nc) as tc:
        with (
            tc.tile_pool(name="persist", bufs=1) as persist,
            tc.tile_pool(name="gwork", bufs=6) as gwork,
            tc.psum_pool(name="psum", bufs=4) as psum,
        ):
            v_t = persist.tile([128, nkc * UPC], f16, tag="v")
            out_sb = persist.tile([BATCH, UPC], f32, tag="o")

            co = 0
            for ci, w in enumerate(CHUNK_COLS):
                fo, fw = co * BATCH, w * BATCH
                g_ts = []
                for kc in range(nkc):
                    g_t = gwork.tile([pkc[kc], fw], f16, tag=f"g{kc}",
                                     name=f"g{ci}_{kc}")
                    nc.sync.dma_start(g_t[:], g_ds[kc][:, fo:fo + fw])
                    g_ts.append(g_t)
                if ci == 0:
                    # v rides behind the first g chunk: off the stream-start
                    # critical path, in SBUF well before the first matmul.
                    nc.sync.dma_start(v_t[:], v_d[:])
                ps = psum.tile([BATCH, w], f32, tag="ps", name=f"ps{ci}")
                # per-column matmuls; the k-chunk pair accumulates in PSUM
                for j in range(w):
                    for kc in range(nkc):
                        nc.tensor.matmul(
                            ps[:, j:j + 1],
                            lhsT=g_ts[kc][:, BATCH * j:BATCH * (j + 1)],
                            rhs=v_t[0:pkc[kc],
                                    kc * UPC + co + j:kc * UPC + co + j + 1],
                            start=(kc == 0),
                            stop=(kc == nkc - 1),
                        )
                nc.scalar.activation(out_sb[:, co:co + w], ps[:],
                                     mybir.ActivationFunctionType.Tanh)
                if ci == FLUSH_AFTER:
                    flushed = co + w
                    nc.scalar.dma_start(out_d[:, 0:flushed],
                                        out_sb[:, 0:flushed])
                co += w
            nc.scalar.dma_start(out_d[:, flushed:UPC],
                                out_sb[:, flushed:UPC])
    nc.compile()
    return nc


def _prepare(x, kernel_vector, bias, nonzero_ind):
    """Host-side shard prep. Returns (kp, per-core input dicts)."""
    x = np.asarray(x, dtype=np.float32)
    v = np.asarray(kernel_vector, dtype=np.float32).ravel()
    bias = np.asarray(bias, dtype=np.float32).ravel()
    ind = np.asarray(nonzero_ind)
    r = ind[:, 0].astype(np.int64)
    c = ind[:, 1].astype(np.int64)

    # COO .set semantics: de-duplicate (row, col), keeping the last occurrence.
    flat = r * UNITS + c
    if len(np.unique(flat)) != len(flat):
        _, last_rev = np.unique(flat[::-1], return_index=True)
        keep = np.sort(len(flat) - 1 - last_rev)
        r, c, v = r[keep], c[keep], v[keep]

    xt16 = np.ascontiguousarray(x.T).astype(np.float16)  # [INPUT_DIM, BATCH]

    # Sort by column, assign each entry its slot k within its column.
    order = np.argsort(c, kind="stable")
    r_s, c_s, v_s = r[order], c[order], v[order]
    counts = np.bincount(c_s, minlength=UNITS)
    kp = int(counts.max()) + 1  # +1: bias slot
    nkc = -(-kp // 128)
    starts = np.zeros(UNITS + 1, dtype=np.int64)
    np.cumsum(counts, out=starts[1:])
    k_s = np.arange(len(c_s), dtype=np.int64) - starts[c_s]

    # g_full[c, k, b]: x row for the entry at (column c, slot k); padding 0.
    g_full = np.zeros((UNITS, kp, BATCH), dtype=np.float16)
    g_full[c_s, k_s] = xt16[r_s]
    v_full = np.zeros((UNITS, kp), dtype=np.float16)
    v_full[c_s, k_s] = v_s.astype(np.float16)
    # bias as one extra entry: value bias[c], "x vector" of ones
    cols = np.arange(UNITS)
    g_full[cols, counts] = np.float16(1.0)
    v_full[cols, counts] = bias.astype(np.float16)

    g_full = g_full.reshape(N_CORES, UPC, kp, BATCH)
    v_full = v_full.reshape(N_CORES, UPC, kp)

    in_maps = []
    for d in range(N_CORES):
        m = {}
        for kc in range(nkc):
            p = min(128, kp - 128 * kc)
            # [UPC, p, B] -> [p, UPC, B] -> [p, UPC*B] (c-major free dim)
            blk = g_full[d, :, 128 * kc:128 * kc + p, :]
            m[f"g{kc}"] = np.ascontiguousarray(
                blk.transpose(1, 0, 2)).reshape(p, UPC * BATCH)
        v_core = np.zeros((128, nkc * UPC), dtype=np.float16)
        for kc in range(nkc):
            p = min(128, kp - 128 * kc)
            v_core[:p, kc * UPC:kc * UPC + UPC] = \
                v_full[d, :, 128 * kc:128 * kc + p].T
        m["vals"] = v_core
        in_maps.append(m)
    return kp, in_maps


def _unshard(res):
    out = np.concatenate(
        [np.asarray(res.results[d]["out"]).reshape(BATCH, UPC)
         for d in range(N_CORES)], axis=1)
    return np.ascontiguousarray(out).astype(np.float32)


def _run(inputs, trace=False):
    from concourse.bass_utils import run_bass_kernel_spmd

    kp, in_maps = _prepare(**inputs)
    if kp not in _PROGRAM_CACHE:
        _PROGRAM_CACHE[kp] = _build_program(kp)
    nc = _PROGRAM_CACHE[kp]
    res = None
    for attempt in range(3):
        try:
            res = run_bass_kernel_spmd(
                nc, in_maps, list(range(N_CORES)), trace=trace,
            )
            break
        except Exception:
            # Transient device faults (e.g. NRT_EXEC_UNIT_UNRECOVERABLE)
            # clear on re-execution; re-raise only if persistent.
            if attempt == 2:
                raise
    assert res is not None
    return _unshard(res), res


def kernel(**inputs):
    out, _ = _run(inputs, trace=False)
    return out


# revision 10
# speedup vs baseline: 1.5642x; 1.0098x over previous
"""Sparse-weight matmul (BiologicalModule) on 8 Trainium2 NeuronCores.

Computes: out = tanh(x @ scatter_coo(kernel_vector, nonzero_ind) + bias)
  x [32, 30000] f32, 500K COO nonzeros into a [30000, 2048] weight matrix.

Strategy (units-sharded, 256 output columns per core):
  - Never materialize the dense [30000, 2048] weight matrix. In CSC view,
    out[b, c] = sum_k v[c,k] * x[r[c,k], b].
  - Host packs a padded-CSC payload with the entry-slot axis k on SBUF
    PARTITIONS: per core, per k-chunk kc, g[k_p, (c, b)] holds the x values
    each entry touches (fp16) and v[k_p, c] the entry values. The bias is
    folded in as one extra entry slot per column (g=1, v=bias[c]), and the
    slot axis is cut exactly at max_count+1 (the last k-chunk has < 128
    partition rows - no padding in the stream).
  - For a single column the multiply+reduce IS one TensorE matmul:
        psum[32b, 1] (+)= g_col[Pk, 32b]^T @ v_col[Pk, 1]
    accumulated over the k-chunks in PSUM. The whole compute runs on the
    otherwise-idle PE engine; ACT applies tanh per chunk of columns
    (PSUM -> SBUF f32); DVE/GPSIMD do nothing.
  - DMA-in streams ~4 MB/core in a few large chunks; per-column matmuls
    are issued in chunk-arrival order (kc0 columns, then kc1 closing the
    PSUM accumulation), so all compute hides under the DMA stream. Column
    chunks shrink toward the end to keep the post-stream tail short.
"""

import sys

import numpy as np

_TRN_REPO = "/opt/trn_rl_repo"
if _TRN_REPO not in sys.path:
    sys.path.insert(0, _TRN_REPO)

INPUT_DIM = 30000
UNITS = 2048
BATCH = 32
N_CORES = 8
UPC = UNITS // N_CORES  # 256 columns per core
FREE = BATCH * UPC  # 8192 free elems per kc
# Columns per DMA chunk. >=8 keeps every DMA line >=512B; shrinking tail
# chunks keep the post-stream critical path short. Chunks in one group
# share a PSUM tile and flush through a single ACT.
CHUNK_GROUPS = [[64], [64], [64], [48], [8, 8]]
assert sum(sum(g) for g in CHUNK_GROUPS) == UPC
# After which group to flush the bulk of the outputs.
FLUSH_AFTER = 2

_PROGRAM_CACHE = {}


def _build_program(kp):
    """Build + compile the SPMD bass program for exact column length kp."""
    from concourse import bacc, tile
    import concourse.mybir as mybir

    f32 = mybir.dt.float32
    f16 = mybir.dt.float16
    nkc = -(-kp // 128)
    pkc = [min(128, kp - 128 * kc) for kc in range(nkc)]

    nc = bacc.Bacc("TRN2", target_bir_lowering=False, debug=False,
                   num_devices=N_CORES)
    g_ds = [nc.dram_tensor(f"g{kc}", [pkc[kc], FREE], f16,
                           kind="ExternalInput") for kc in range(nkc)]
    v_d = nc.dram_tensor("vals", [128, nkc * UPC], f16, kind="ExternalInput")
    out_d = nc.dram_tensor("out", [BATCH, UPC], f32, kind="ExternalOutput")

    with tile.TileContext(nc) as tc:
        with (
            tc.tile_pool(name="persist", bufs=1) as persist,
            tc.tile_pool(name="gwork", bufs=6) as gwork,
            tc.psum_pool(name="psum", bufs=4) as psum,
        ):
            v_t = persist.tile([128, nkc * UPC], f16, tag="v")
            out_sb = persist.tile([BATCH, UPC], f32, tag="o")

            co = 0
            ci = 0
            for gi, group in enumerate(CHUNK_GROUPS):
                gw = sum(group)
                ps = psum.tile([BATCH, gw], f32, tag="ps", name=f"ps{gi}")
                go = co
                for w in group:
                    fo, fw = co * BATCH, w * BATCH
                    g_ts = []
                    for kc in range(nkc):
                        g_t = gwork.tile([pkc[kc], fw], f16, tag=f"g{kc}",
                                         name=f"g{ci}_{kc}")
                        nc.sync.dma_start(g_t[:], g_ds[kc][:, fo:fo + fw])
                        g_ts.append(g_t)
                    if ci == 0:
                        # v rides behind the first g chunk: off the
                        # stream-start critical path, in SBUF well before
                        # the first matmul.
                        nc.sync.dma_start(v_t[:], v_d[:])
                    # per-column matmuls; the k-chunk pair accumulates in PSUM
                    for j in range(w):
                        pj = co - go + j
                        for kc in range(nkc):
                            nc.tensor.matmul(
                                ps[:, pj:pj + 1],
                                lhsT=g_ts[kc][:, BATCH * j:BATCH * (j + 1)],
                                rhs=v_t[0:pkc[kc],
                                        kc * UPC + co + j:
                                        kc * UPC + co + j + 1],
                                start=(kc == 0),
                                stop=(kc == nkc - 1),
                            )
                    co += w
                    ci += 1
                nc.scalar.activation(out_sb[:, go:go + gw], ps[:],
                                     mybir.ActivationFunctionType.Tanh)
                if gi == FLUSH_AFTER:
                    flushed = co
                    nc.scalar.dma_start(out_d[:, 0:flushed],
                                        out_sb[:, 0:flushed])
            nc.sync.dma_start(out_d[:, flushed:UPC],
                              out_sb[:, flushed:UPC])
    nc.compile()
    return nc


def _prepare(x, kernel_vector, bias, nonzero_ind):
    """Host-side shard prep. Returns (kp, per-core input dicts)."""
    x = np.asarray(x, dtype=np.float32)
    v = np.asarray(kernel_vector, dtype=np.float32).ravel()
    bias = np.asarray(bias, dtype=np.float32).ravel()
    ind = np.asarray(nonzero_ind)
    r = ind[:, 0].astype(np.int64)
    c = ind[:, 1].astype(np.int64)

    # COO .set semantics: de-duplicate (row, col), keeping the last occurrence.
    flat = r * UNITS + c
    if len(np.unique(flat)) != len(flat):
        _, last_rev = np.unique(flat[::-1], return_index=True)
        keep = np.sort(len(flat) - 1 - last_rev)
        r, c, v = r[keep], c[keep], v[keep]

    xt16 = np.ascontiguousarray(x.T).astype(np.float16)  # [INPUT_DIM, BATCH]

    # Sort by column, assign each entry its slot k within its column.
    order = np.argsort(c, kind="stable")
    r_s, c_s, v_s = r[order], c[order], v[order]
    counts = np.bincount(c_s, minlength=UNITS)
    kp = int(counts.max()) + 1  # +1: bias slot
    nkc = -(-kp // 128)
    starts = np.zeros(UNITS + 1, dtype=np.int64)
    np.cumsum(counts, out=starts[1:])
    k_s = np.arange(len(c_s), dtype=np.int64) - starts[c_s]

    # g_full[c, k, b]: x row for the entry at (column c, slot k); padding 0.
    g_full = np.zeros((UNITS, kp, BATCH), dtype=np.float16)
    g_full[c_s, k_s] = xt16[r_s]
    v_full = np.zeros((UNITS, kp), dtype=np.float16)
    v_full[c_s, k_s] = v_s.astype(np.float16)
    # bias as one extra entry: value bias[c], "x vector" of ones
    cols = np.arange(UNITS)
    g_full[cols, counts] = np.float16(1.0)
    v_full[cols, counts] = bias.astype(np.float16)

    g_full = g_full.reshape(N_CORES, UPC, kp, BATCH)
    v_full = v_full.reshape(N_CORES, UPC, kp)

    in_maps = []
    for d in range(N_CORES):
        m = {}
        for kc in range(nkc):
            p = min(128, kp - 128 * kc)
            # [UPC, p, B] -> [p, UPC, B] -> [p, UPC*B] (c-major free dim)
            blk = g_full[d, :, 128 * kc:128 * kc + p, :]
            m[f"g{kc}"] = np.ascontiguousarray(
                blk.transpose(1, 0, 2)).reshape(p, UPC * BATCH)
        v_core = np.zeros((128, nkc * UPC), dtype=np.float16)
        for kc in range(nkc):
            p = min(128, kp - 128 * kc)
            v_core[:p, kc * UPC:kc * UPC + UPC] = \
                v_full[d, :, 128 * kc:128 * kc + p].T
        m["vals"] = v_core
        in_maps.append(m)
    return kp, in_maps


def _unshard(res):
    out = np.concatenate(
        [np.asarray(res.results[d]["out"]).reshape(BATCH, UPC)
         for d in range(N_CORES)], axis=1)
    return np.ascontiguousarray(out).astype(np.float32)


def _run(inputs, trace=False):
    from concourse.bass_utils import run_bass_kernel_spmd

    kp, in_maps = _prepare(**inputs)
    if kp not in _PROGRAM_CACHE:
        _PROGRAM_CACHE[kp] = _build_program(kp)
    nc = _PROGRAM_CACHE[kp]
    res = None
    for attempt in range(3):
        try:
            res = run_bass_kernel_spmd(
                nc, in_maps, list(range(N_CORES)), trace=trace,
            )
            break
        except Exception:
            # Transient device faults (e.g. NRT_EXEC_UNIT_UNRECOVERABLE)
            # clear on re-execution; re-raise only if persistent.
            if attempt == 2:
                raise
    assert res is not None
    return _unshard(res), res


def kernel(**inputs):
    out, _ = _run(inputs, trace=False)
    return out
